# revision 27
# baseline (speedup 1.0000x reference)
"""Trainium2 Bass kernel: Brill-Lindquist Christoffel symbols.

Math: the Brill-Lindquist 3-metric is conformally flat, G = psi^4 * I with
psi(y) = 1 + sum_p m_p / (2 |y - c_p|).  The Christoffel symbols collapse to

    out[b, k, i, j] = E_j d_ik + E_i d_jk - E_k d_ij,
    E_k = 2 * (d_k psi) / psi,
    d_k psi = sum_p (m_p / 2) * (c_pk - x_k) / r_p^3.

(The reference computes d_k(psi^4) by central differences with eps=1e-4; the
analytic derivative agrees with it to well below the reference's own fp32
rounding noise.)

Per point, only 3 values E_0..E_2 exist; they are scattered (with signs) into
21 of the 27 output channels, 6 channels are exactly zero.

Sharding: pure data parallel over the batch across 8 NeuronCores; masses /
centers are folded on the host into a tiny replicated constant vector.
"""

import numpy as np

import concourse.bacc as bacc
import concourse.bass as bass
import concourse.mybir as mybir
import concourse.tile as tile
from concourse.bass_utils import run_bass_kernel_spmd

# The kernel only uses Square/Ln/Exp/Copy/Identity, which all live in the
# combined "natural_log_exp_and_others" ACT table set.  Left to itself the
# table-load pass alternates between the Ln-only and Exp-only sets (one
# ~2.7us ACT_TABLE_LOAD per switch); restricting the choice to the combined
# set yields a single load.
_orig_get_activation_tables = bacc.get_activation_tables


def _prefer_combined_act_tables(arch):
    # Preserve the canonical set list (act_func_set_id is positional against
    # act_info.json) but strip this kernel's functions from every other set,
    # so the load-insertion pass resolves them all to the one combined set.
    tables = _orig_get_activation_tables(arch)
    mine = {
        mybir.ActivationFunctionType.Square,
        mybir.ActivationFunctionType.Ln,
        mybir.ActivationFunctionType.Exp,
        mybir.ActivationFunctionType.Copy,
        mybir.ActivationFunctionType.Identity,
    }
    outtables = {}
    for name, funcs in tables.items():
        if name == "natural_log_exp_and_others":
            outtables[name] = funcs
        else:
            outtables[name] = set(funcs) - mine
    return outtables


import os as _os
if _os.environ.get("BL_NO_TABLE_FIX") != "1":
    bacc.get_activation_tables = _prefer_combined_act_tables

AF = mybir.ActivationFunctionType
OP = mybir.AluOpType
AX = mybir.AxisListType
F32 = mybir.dt.float32

N_CORES = 8
P = 128
TILE_T = 128  # points per partition per steady tile

DEFAULT_FEATURES = frozenset({"joint_dx", "stt"})


def build_program(bc: int, tile_t: int = TILE_T, reps: int = 1,
                  features: frozenset = DEFAULT_FEATURES):
    """Bass program for one core processing `bc` points (SPMD across cores).

    reps > 1 repeats the whole tile loop (same data, same outputs) — used only
    by the perf harness to measure steady-state kernel time above the
    constant dispatch overhead.
    """
    joint_dx = "joint_dx" in features
    stt = "stt" in features

    npp = bc // P
    T = tile_t
    ntiles = npp // T
    assert bc == npp * P and npp == ntiles * T
    sched = []
    pos = 0
    if npp == 1024 and T == 256:
        # latency ramp: small leading tiles start the store queue early
        for sz in (64, 192):
            sched.append((pos, sz))
            pos += sz
    while pos < npp:
        sz = min(T, npp - pos)
        sched.append((pos, sz))
        pos += sz

    nc = bacc.Bacc(None, target_bir_lowering=False)
    x = nc.dram_tensor("x", [bc, 3], F32, kind="ExternalInput")
    consts = nc.dram_tensor("consts", [P, 16], F32, kind="ExternalInput")
    out = nc.dram_tensor("out", [bc, 27], F32, kind="ExternalOutput")
    xv = x.rearrange("(p n) k -> p n k", p=P)
    ov = out.rearrange("(p n) c -> p n c", p=P)

    def chans(ot, base, step, count, T):
        # [P, T, count] view over output channels {base, base+step, ...}
        return bass.AP(
            tensor=ot.tensor,
            offset=ot.offset + base,
            ap=[ot.ap[0], [27, T], [step, count]],
        )

    with tile.TileContext(nc) as tc:
        with (
            tc.tile_pool(name="cpool", bufs=1) as cpool,
            tc.tile_pool(name="xpool", bufs=4) as xpool,
            tc.tile_pool(name="opool", bufs=1) as opool,
            tc.tile_pool(name="wpool", bufs=2) as wpool,
        ):
            # warmup: pull the one-time ACT table load off the critical path
            warm = cpool.tile([P, 8], F32)
            nc.vector.memset(warm[:], 1.0)
            wz = cpool.tile([P, 1], F32)
            nc.vector.memset(wz[:], 0.0)
            warm2 = cpool.tile([P, 8], F32)
            nc.scalar.activation(warm2[:], warm[:], AF.Square, bias=wz[:])

            ct = cpool.tile([P, 16], F32)
            nc.sync.dma_start(out=ct[:], in_=consts[:])

            n_ot = min(4, len(sched))
            otiles = []
            for j in range(n_ot):
                sizes = [sched[k][1] for k in range(j, len(sched), n_ot)]
                Tj = max(sizes)
                ot = opool.tile([P, Tj, 27], F32, tag=f"ot{j}", name=f"ot{j}")
                otiles.append(ot)
            ot_zeroed = set()

            def zero_channels(ot, Tj):
                nc.gpsimd.memset(chans(ot, 5, 16, 2, Tj), 0.0)
                nc.gpsimd.memset(chans(ot, 7, 4, 4, Tj), 0.0)

            for step in range(len(sched) * reps):
                i = step % len(sched)
                start, T = sched[i]
                xin = xpool.tile([P, T, 3], F32, tag="xin", name="xin")
                # input loads ride the ACT HWDGE queue; output stores the SP
                # queue — independent FIFOs
                nc.scalar.dma_start(out=xin[:], in_=xv[:, start:start + T, :])
                xin = xin[:]

                if joint_dx:
                    dxj = wpool.tile([P, T, 6], F32, name="dxj")
                    xin2 = bass.AP(
                        tensor=xin.tensor, offset=xin.offset,
                        ap=[xin.ap[0], [3, T], [0, 2], [1, 3]],
                    )
                    cb = ct[:, 0:6].unsqueeze(1).broadcast_to([P, T, 6])
                    dxv = bass.AP(
                        tensor=dxj.tensor, offset=dxj.offset,
                        ap=[dxj.ap[0], [6, T], [3, 2], [1, 3]],
                    )
                    nc.vector.tensor_tensor(
                        dxv, cb.rearrange("p t (c k) -> p t c k", k=3), xin2,
                        OP.subtract,
                    )
                    dx0 = dxj[:, :, 0:3]
                    dx1 = dxj[:, :, 3:6]
                    sqj = wpool.tile([P, T, 6], F32, name="sqj")
                    nc.scalar.activation(sqj[:], dxj[:], AF.Square, bias=ct[:, 10:11])
                    sq0 = sqj[:, :, 0:3]
                    sq1 = sqj[:, :, 3:6]
                else:
                    c0b = ct[:, 0:3].unsqueeze(1).broadcast_to([P, T, 3])
                    c1b = ct[:, 3:6].unsqueeze(1).broadcast_to([P, T, 3])
                    dx0t = wpool.tile([P, T, 3], F32, name="dx0t")
                    nc.vector.tensor_tensor(dx0t[:], c0b, xin[:], OP.subtract)
                    dx1t = wpool.tile([P, T, 3], F32, name="dx1t")
                    nc.vector.tensor_tensor(dx1t[:], c1b, xin[:], OP.subtract)
                    dx0, dx1 = dx0t[:], dx1t[:]
                    sq0t = wpool.tile([P, T, 3], F32, name="sq0t")
                    nc.scalar.activation(sq0t[:], dx0, AF.Square, bias=ct[:, 10:11])
                    sq1t = wpool.tile([P, T, 3], F32, name="sq1t")
                    nc.scalar.activation(sq1t[:], dx1, AF.Square, bias=ct[:, 10:11])
                    sq0, sq1 = sq0t[:], sq1t[:]

                s0 = wpool.tile([P, T], F32, name="s0")
                nc.vector.tensor_reduce(s0[:], sq0, axis=AX.X, op=OP.add)
                s1 = wpool.tile([P, T], F32, name="s1")
                nc.vector.tensor_reduce(s1[:], sq1, axis=AX.X, op=OP.add)

                # ln_p = Ln(g_p s_p), g_p = (m_p/2)^(-2/3):
                #   u_p := (g_p s_p)^-1/2 = (m_p/2)^(1/3) / r_p
                #   t_p := (g_p s_p)^-3/2 = (m_p/2) / r_p^3
                ln0 = wpool.tile([P, T], F32, name="ln0")
                nc.scalar.activation(ln0[:], s0[:], AF.Ln, bias=ct[:, 10:11], scale=ct[:, 6:7])
                ln1 = wpool.tile([P, T], F32, name="ln1")
                nc.scalar.activation(ln1[:], s1[:], AF.Ln, bias=ct[:, 10:11], scale=ct[:, 7:8])
                u0 = wpool.tile([P, T], F32, name="u0")
                nc.scalar.activation(u0[:], ln0[:], AF.Exp, bias=ct[:, 10:11], scale=-0.5)
                u1 = wpool.tile([P, T], F32, name="u1")
                nc.scalar.activation(u1[:], ln1[:], AF.Exp, bias=ct[:, 10:11], scale=-0.5)
                t0 = wpool.tile([P, T], F32, name="t0")
                nc.scalar.activation(t0[:], ln0[:], AF.Exp, bias=ct[:, 10:11], scale=-1.5)
                t1 = wpool.tile([P, T], F32, name="t1")
                nc.scalar.activation(t1[:], ln1[:], AF.Exp, bias=ct[:, 10:11], scale=-1.5)

                # psi - 1 = q0 u0 + q1 u1 with q_p = (m_p/2)^(2/3)
                v = wpool.tile([P, T], F32, name="v")
                if stt:
                    h = wpool.tile([P, T], F32, name="h")
                    nc.vector.tensor_scalar(h[:], u1[:], ct[:, 9:10], None, OP.mult)
                    nc.vector.scalar_tensor_tensor(
                        v[:], u0[:], ct[:, 8:9], h[:], OP.mult, OP.add
                    )
                else:
                    h0 = wpool.tile([P, T], F32, name="h0")
                    nc.vector.tensor_scalar(h0[:], u0[:], ct[:, 8:9], None, OP.mult)
                    h1 = wpool.tile([P, T], F32, name="h1")
                    nc.vector.tensor_scalar(h1[:], u1[:], ct[:, 9:10], None, OP.mult)
                    nc.vector.tensor_tensor(v[:], h0[:], h1[:], OP.add)
                lnp = wpool.tile([P, T], F32, name="lnp")
                nc.scalar.activation(lnp[:], v[:], AF.Ln, bias=ct[:, 11:12])
                w2 = wpool.tile([P, T], F32, name="w2")
                nc.scalar.activation(w2[:], lnp[:], AF.Exp, bias=ct[:, 12:13], scale=-1.0)

                # grad_k = sum_p (m_p/2) r_p^-3 (c_p - x)_k = d_k psi
                mul0 = wpool.tile([P, T, 3], F32, name="mul0")
                nc.vector.tensor_tensor(
                    mul0[:], dx0, t0.unsqueeze(2).broadcast_to([P, T, 3]), OP.mult
                )
                mul1 = wpool.tile([P, T, 3], F32, name="mul1")
                nc.vector.tensor_tensor(
                    mul1[:], dx1, t1.unsqueeze(2).broadcast_to([P, T, 3]), OP.mult
                )
                grad = wpool.tile([P, T, 3], F32, name="grad")
                nc.vector.tensor_tensor(grad[:], mul0[:], mul1[:], OP.add)

                ot_full = otiles[i % n_ot]
                if i % n_ot not in ot_zeroed:
                    ot_zeroed.add(i % n_ot)
                    zero_channels(ot_full, ot_full.shape[1])
                ot = ot_full[:, 0:T, :]
                # [E0,E1,E2] -> channels {0,1,2}; replicate to {12..14}, {24..26}
                nc.vector.tensor_tensor(
                    chans(ot, 0, 1, 3, T),
                    grad[:],
                    w2.unsqueeze(2).broadcast_to([P, T, 3]),
                    OP.mult,
                )
                nc.scalar.copy(chans(ot, 12, 1, 3, T), chans(ot, 0, 1, 3, T))
                nc.scalar.copy(chans(ot, 24, 1, 3, T), chans(ot, 0, 1, 3, T))
                E0 = chans(ot, 0, 0, 2, T)
                E1 = chans(ot, 1, 0, 2, T)
                E2 = chans(ot, 2, 0, 2, T)
                # remaining 12 nonzero channels as 6 paired strided copies
                nc.scalar.mul(chans(ot, 4, 4, 2, T), E0, -1.0)
                nc.scalar.copy(chans(ot, 10, 10, 2, T), E0)
                nc.scalar.mul(chans(ot, 18, 4, 2, T), E2, -1.0)
                nc.scalar.copy(chans(ot, 3, 20, 2, T), E1)
                nc.scalar.mul(chans(ot, 9, 8, 2, T), E1, -1.0)
                nc.scalar.copy(chans(ot, 6, 10, 2, T), E2)

                nc.sync.dma_start(out=ov[:, start:start + T, :], in_=ot)
    nc.compile()
    return nc


_programs: dict = {}


def _get_program(bc: int):
    if bc not in _programs:
        _programs[bc] = build_program(bc)
    return _programs[bc]


def make_consts(masses: np.ndarray, centers: np.ndarray) -> np.ndarray:
    m = np.asarray(masses, dtype=np.float64)
    c = np.asarray(centers, dtype=np.float64)
    g = (m / 2.0) ** (-2.0 / 3.0)
    q = (m / 2.0) ** (2.0 / 3.0)
    vec = np.concatenate(
        [c[0], c[1], g, q, [0.0, 1.0, np.log(2.0)], m / 2.0, [0.0]]
    ).astype(np.float32)  # [16]
    return np.tile(vec[None, :], (P, 1))


def kernel(x, masses, centers):
    x = np.ascontiguousarray(np.asarray(x, dtype=np.float32))
    B = x.shape[0]
    assert B % N_CORES == 0
    bc = B // N_CORES
    nc = _get_program(bc)
    consts = make_consts(masses, centers)
    in_maps = [
        {"x": x[i * bc:(i + 1) * bc], "consts": consts} for i in range(N_CORES)
    ]
    res = run_bass_kernel_spmd(nc, in_maps, core_ids=list(range(N_CORES)))
    full = np.concatenate([res.results[i]["out"] for i in range(N_CORES)], axis=0)
    return full.reshape(B, 3, 3, 3)
